# revision 19
# baseline (speedup 1.0000x reference)
"""Trainium2 Bass kernel for nn_Conv4D: 4D conv with separable 3x3x3x3 kernel.

Math: for each batch b, with X[b] = x[b].reshape(64, 64) (rows = (d1,d2) flat,
cols = (d3,d4) flat):

    out[b] = sum_{c,d in 3x3} (K[c,d] * W)^T @ X[b][:, window(c,d)]
           =  W^T @ ( sum_{c,d} K[c,d] * X[b][:, window(c,d)] )

where W[ (i'+a)*8 + (j'+e), i'*6+j' ] = K[a,e] is the 64->36 banded matrix of
the (d1,d2)-conv and window(c,d) the shifted 6x6 (d3,d4) patch.  Two
equivalent schedules, used side by side on disjoint supergroups to load-
balance the Tensor and Vector engines:

  PE path : 9 PSUM-accumulated matmuls per group against shifted free-dim
            views (stationary = K[c,d]*W block-diag stacks).  324 PE
            rows/pair, DVE idle.
  DVE path: the kl-conv is an elementwise free-dim stencil z = sum K[cd] *
            shift(x) -- 9 Vector-engine taps (tensor_scalar_mul +
            affine_then_add, f32 accumulation, last tap emits bf16) --
            followed by ONE matmul per group (stationary = plain W).  36 PE
            rows/pair, 324 DVE elem-cycles/pair.

Batch packing: 2 batches stack on the 128 partitions (partition = 64*h + ij).
Block-diagonal weights [ [Wcd, 0], [0, Wcd] ] (128x72) route each batch's 64
ij-rows to its own 36 output partitions.  K=128, M=72, N = 14 pairs * 36 =
504 <= 512 (moving-operand max; one PSUM bank).

DMA layout: host-side prep is free, so the input is pre-shuffled ON HOST into
the exact SBUF image and downcast to bf16 (tolerance 2e-2; bf16 costs ~4e-3):
per-partition DMA runs are multi-KB (vs 256B rows -> packet-bound at 120GB/s
in the naive layout).  Within a supergroup the image is [p][k(8)][l(8)][n]
with the pair index n innermost.  The bf16 output image [72, PAIRS*36] is
un-shuffled + upcast on host.  Kernel scalars K[c,d] ride in an SBUF tensor
(per-partition scale APs), NOT as baked immediates, so the compiled program
is input-independent.

Sharding: pure data parallelism, batch dim split across 8 cores (1024 each).
"""

import numpy as np
import ml_dtypes

import concourse.bass as bass
import concourse.bacc as bacc
import concourse.mybir as mybir
from concourse.tile import TileContext
from concourse.bass_utils import run_bass_kernel_spmd

N_CORES = 8
B = 8192
B_C = B // N_CORES            # 1024 batches per core
PAIRS = B_C // 2              # 512 batch pairs per core
PAIRS_PER_GROUP = 14          # N = 14*36 = 504 <= 512 (moving-operand max)
GPG = 4                       # groups per supergroup (one in-DMA / out-DMA)
SUPER = GPG * PAIRS_PER_GROUP # 56 pairs = 112 batches
BF16 = mybir.dt.bfloat16
F32 = mybir.dt.float32
NP_BF16 = ml_dtypes.bfloat16

SHIFTS = [(c, d) for c in range(3) for d in range(3)]

# Per-supergroup engine assignment: 'P' = 9-matmul PE path, 'D' = DVE
# stencil path.  Alternate/ratio chosen to balance PE vs DVE occupancy.
ASSIGN = "PDPDPDPDPD"

# The stencil pads l' to the full 8 so (l,n) flattens to one stride-1 dim
# (custom DVE ops allow only 2 free dims); l'=6,7 columns are garbage and
# never read.  The widest tap (c=2,d=2) then reads 2*S columns past the
# supergroup's 64*S image -- XPAD junk columns at the end of x cover it.
XPAD = 2 * SUPER


def _super_sizes():
    sizes = []
    left = PAIRS
    while left > 0:
        n = min(SUPER, left)
        sizes.append(n)
        left -= n
    return sizes


def build_w_stack(kern: np.ndarray) -> np.ndarray:
    """9 block-diagonal K[cd]*W stacks + 1 plain-W stack + K scales,
    concatenated as one [128, 9*72 + 72 + 9] bf16 weights image."""
    kern = np.asarray(kern, np.float32)
    W = np.zeros((64, 36), np.float32)
    for ip in range(6):
        for jp in range(6):
            m = ip * 6 + jp
            for a in range(3):
                for e in range(3):
                    W[(ip + a) * 8 + (jp + e), m] = kern[a, e]
    wstack = np.zeros((128, 9 * 72 + 72), np.float32)
    for s, (c, d) in enumerate(SHIFTS):
        wcd = kern[c, d] * W
        wstack[0:64, s * 72 : s * 72 + 36] = wcd
        wstack[64:128, s * 72 + 36 : s * 72 + 72] = wcd
    wstack[0:64, 648:684] = W
    wstack[64:128, 684:720] = W
    # K[cd] scales replicated per partition, f32 (tensor_scalar requires it)
    kscal = np.broadcast_to(kern.reshape(1, 9), (128, 9)).copy()
    return wstack.astype(NP_BF16), kscal


_PROGRAM_CACHE = {}


def build_program() -> bass.Bass:
    if "nc" in _PROGRAM_CACHE:
        return _PROGRAM_CACHE["nc"]

    # Bacc (not raw Bass): its compile()/finalize() runs
    # move_matmul_waits_to_ldweights + generate_event_semaphores, which split
    # multi-wait instructions (TRN2 allows 1 sync wait per instruction).
    nc = bacc.Bacc()
    x = nc.dram_tensor("x", [128, PAIRS * 64 + XPAD], BF16, kind="ExternalInput")
    w = nc.dram_tensor("w", [128, 9 * 72 + 72], BF16, kind="ExternalInput")
    k = nc.dram_tensor("k", [128, 9], F32, kind="ExternalInput")
    o = nc.dram_tensor("o", [72, PAIRS * 36], BF16, kind="ExternalOutput")

    with TileContext(nc) as tc:
        with (
            tc.tile_pool(name="wp", bufs=1) as wp,
            tc.tile_pool(name="xp", bufs=3) as xp,
            tc.tile_pool(name="zp", bufs=2) as zp,
            tc.tile_pool(name="pp", bufs=6, space="PSUM") as pp,
            tc.tile_pool(name="op", bufs=3) as op,
        ):
            wt = wp.tile([128, 9 * 72 + 72], BF16)
            nc.sync.dma_start(out=wt[:, :], in_=w[:, :])
            ktile = wp.tile([128, 9], F32)
            nc.sync.dma_start(out=ktile[:, :], in_=k[:, :])
            wplain = wt[:, 648:720]
            kt = ktile

            gidx = 0
            pcur = 0  # pair cursor
            for sidx, spairs in enumerate(_super_sizes()):
                path = ASSIGN[sidx % len(ASSIGN)]
                xg = xp.tile([128, SUPER * 64 + XPAD], BF16, tag="xg")
                nc.sync.dma_start(
                    out=xg[:, : spairs * 64 + XPAD],
                    in_=x[:, pcur * 64 : (pcur + spairs) * 64 + XPAD],
                )
                ot = op.tile([72, SUPER * 36], BF16, tag="ot")
                # SBUF image within a supergroup: [p][k(8)][l(8)][n].
                xv = xg[:, : spairs * 64].rearrange(
                    "p (k l n) -> p k l n", k=8, l=8
                )

                if path == "D":
                    # z[p, k', l8, n] = sum_cd K[cd] * x[p, k'+c, l8+d, n],
                    # l8 padded to 8 (l8=6,7 garbage, never read).  Taps are
                    # rank-3: [p][k'(6)][flat (l,n) 8S], bf16 accumulation.
                    z16 = zp.tile([128, SUPER * 48], BF16, tag="z16")
                    S = spairs
                    zt = z16[:, : 48 * S].rearrange("p (k f) -> p k f", k=6)
                    for s, (c, d) in enumerate(SHIFTS):
                        # rows k'+c of the 8x8 image, cols l8+d wrapped:
                        # flat offset (8c+d)*S, 6 rows of stride 8S, 8S run
                        xw = xg[
                            :, (8 * c + d) * S : (8 * c + d) * S + 48 * S
                        ].rearrange("p (k f) -> p k f", k=6)
                        if s == 0:
                            nc.vector.tensor_scalar_mul(zt, xw, kt[:, 0:1])
                        else:
                            nc.vector.affine_then_add(
                                zt, xw, zt, kt[:, s : s + 1], 0.0
                            )
                    zmm = z16[:, : 48 * S].rearrange(
                        "p (k l n) -> p k l n", k=6, l=8
                    )

                done = 0
                while done < spairs:
                    npair = min(PAIRS_PER_GROUP, spairs - done)
                    nfree = npair * 36

                    ps = pp.tile([72, PAIRS_PER_GROUP * 36], F32, tag="ps")
                    # Gate matmul: absorbs the psum-slot-release (and, for
                    # group 0, the weight-DMA) wait so each real matmul
                    # carries at most one sync wait.
                    nc.tensor.matmul(
                        ps[0:2, 0:2], wt[:, 0:2], wt[:, 0:2], start=True, stop=True
                    )
                    if path == "P":
                        for s, (c, d) in enumerate(SHIFTS):
                            nc.tensor.matmul(
                                ps[:, :nfree],
                                wt[:, s * 72 : (s + 1) * 72],
                                xv[:, c : c + 6, d : d + 6, done : done + npair],
                                start=(s == 0),
                                stop=(s == len(SHIFTS) - 1),
                            )
                    else:
                        nc.tensor.matmul(
                            ps[:, :nfree],
                            wplain,
                            zmm[:, :, 0:6, done : done + npair],
                            start=True,
                            stop=True,
                        )

                    dst = ot[:, done * 36 : done * 36 + nfree]
                    nc.scalar.copy(out=dst, in_=ps[:, :nfree])
                    done += npair
                    gidx += 1

                nc.sync.dma_start(
                    out=o[:, pcur * 36 : (pcur + spairs) * 36],
                    in_=ot[:, : spairs * 36],
                )
                pcur += spairs

    # Bacc.finalize runs compile() (register alloc, wait splitting via event
    # semaphores) then freezes; the PJRT exec path requires a finalized nc.
    nc.finalize()

    _PROGRAM_CACHE["nc"] = nc
    return nc


def shard_inputs(input_tensor: np.ndarray, kern: np.ndarray):
    """Host prep: shuffle each core's slice into the SBUF image and downcast.

    Per supergroup of S pairs starting at pair P: partition p = 64*h + ij
    holds x[2*(P+n)+h, ij, k*8+l] at free offset P*64 + (k*8+l)*S + n.
    """
    x = np.ascontiguousarray(np.asarray(input_tensor, np.float32))
    xs = x.reshape(N_CORES, PAIRS, 2, 64, 64)  # (core, pair, h, ij, kl)
    wstack, kscal = build_w_stack(kern)
    in_maps = []
    for c in range(N_CORES):
        blocks = []
        pcur = 0
        for spairs in _super_sizes():
            blk = xs[c][pcur : pcur + spairs]          # (S, 2, 64, 64)
            blk = blk.transpose(1, 2, 3, 0)            # (2, ij, kl, S)
            blocks.append(blk.reshape(128, spairs * 64))
            pcur += spairs
        blocks.append(np.zeros((128, XPAD), np.float32))
        xd = np.concatenate(blocks, axis=1).astype(NP_BF16)
        in_maps.append({"x": np.ascontiguousarray(xd), "w": wstack, "k": kscal})
    return in_maps


def unshard_output(results) -> np.ndarray:
    """o[36*h+ij', (P+done)*36 + kl'*npair + n] -> out[b, i',j',k',l']."""
    outs = []
    for r in results:
        od = np.asarray(r["o"]).astype(np.float32)  # (72, PAIRS*36)
        out = np.empty((B_C, 36, 36), np.float32)
        pcur = 0
        for spairs in _super_sizes():
            done = 0
            while done < spairs:
                npair = min(PAIRS_PER_GROUP, spairs - done)
                col0 = (pcur + done) * 36
                blk = od[:, col0 : col0 + npair * 36]
                # rows (2, 36) x cols (36, npair) -> (npair, 2, ij', kl')
                blk = blk.reshape(2, 36, 36, npair).transpose(3, 0, 1, 2)
                b0 = (pcur + done) * 2
                out[b0 : b0 + 2 * npair] = blk.reshape(2 * npair, 36, 36)
                done += npair
            pcur += spairs
        outs.append(out.reshape(B_C, 6, 6, 6, 6))
    return np.concatenate(outs, axis=0)


def run(input_tensor: np.ndarray, kern: np.ndarray, **spmd_kwargs):
    """Shard, run on 8 cores, gather.  Returns (output, BassKernelResults)."""
    in_maps = shard_inputs(input_tensor, kern)
    nc = build_program()
    res = run_bass_kernel_spmd(nc, in_maps, core_ids=list(range(N_CORES)), **spmd_kwargs)
    return unshard_output(res.results), res


def kernel(input_tensor: np.ndarray, kernel: np.ndarray) -> np.ndarray:
    out, _ = run(input_tensor, kernel)
    return out


# revision 21
# speedup vs baseline: 1.3630x; 1.3630x over previous
"""Trainium2 Bass kernel for nn_Conv4D: 4D conv with separable 3x3x3x3 kernel.

Math: for each batch b, with X[b] = x[b].reshape(64, 64) (rows = (d1,d2) flat,
cols = (d3,d4) flat):

    out[b] = sum_{c,d in 3x3} (K[c,d] * W)^T @ X[b][:, window(c,d)]
           =  W^T @ ( sum_{c,d} K[c,d] * X[b][:, window(c,d)] )

where W[ (i'+a)*8 + (j'+e), i'*6+j' ] = K[a,e] is the 64->36 banded matrix of
the (d1,d2)-conv and window(c,d) the shifted 6x6 (d3,d4) patch.  Two
equivalent schedules, used side by side on disjoint supergroups to load-
balance the Tensor and Vector engines:

  PE path : 9 PSUM-accumulated matmuls per group against shifted free-dim
            views (stationary = K[c,d]*W block-diag stacks).  324 PE
            rows/pair, DVE idle.
  DVE path: the kl-conv is an elementwise free-dim stencil z = sum K[cd] *
            shift(x) -- 9 Vector-engine taps (tensor_scalar_mul +
            affine_then_add, f32 accumulation, last tap emits bf16) --
            followed by ONE matmul per group (stationary = plain W).  36 PE
            rows/pair, 324 DVE elem-cycles/pair.

Batch packing: 2 batches stack on the 128 partitions (partition = 64*h + ij).
Block-diagonal weights [ [Wcd, 0], [0, Wcd] ] (128x72) route each batch's 64
ij-rows to its own 36 output partitions.  K=128, M=72, N = 14 pairs * 36 =
504 <= 512 (moving-operand max; one PSUM bank).

DMA layout: host-side prep is free, so the input is pre-shuffled ON HOST into
the exact SBUF image and downcast to bf16 (tolerance 2e-2; bf16 costs ~4e-3):
per-partition DMA runs are multi-KB (vs 256B rows -> packet-bound at 120GB/s
in the naive layout).  Within a supergroup the image is [p][k(8)][l(8)][n]
with the pair index n innermost.  The bf16 output image [72, PAIRS*36] is
un-shuffled + upcast on host.  Kernel scalars K[c,d] ride in an SBUF tensor
(per-partition scale APs), NOT as baked immediates, so the compiled program
is input-independent.

Sharding: pure data parallelism, batch dim split across 8 cores (1024 each).
"""

import numpy as np
import ml_dtypes

import concourse.bass as bass
import concourse.bacc as bacc
import concourse.mybir as mybir
from concourse.tile import TileContext
from concourse.bass_utils import run_bass_kernel_spmd

N_CORES = 8
B = 8192
B_C = B // N_CORES            # 1024 batches per core
PAIRS = B_C // 2              # 512 batch pairs per core
PAIRS_PER_GROUP = 14          # N = 14*36 = 504 <= 512 (moving-operand max)
GPG = 4                       # groups per supergroup (one in-DMA / out-DMA)
SUPER = GPG * PAIRS_PER_GROUP # 56 pairs = 112 batches
BF16 = mybir.dt.bfloat16
F32 = mybir.dt.float32
NP_BF16 = ml_dtypes.bfloat16

SHIFTS = [(c, d) for c in range(3) for d in range(3)]

# Per-supergroup engine assignment: 'P' = 9-matmul PE path, 'D' = DVE
# stencil path.  Ratio balances PE (~7.6us/P-super) vs DVE (~13.6us/D-super);
# the pattern places each D late enough that its stencil is already done when
# the PE's in-order stream reaches that super's matmuls (no head-of-line
# stall).
ASSIGN = "PPDPPDPPDD"

# The stencil pads l' to the full 8 so (l,n) flattens to one stride-1 dim
# (custom DVE ops allow only 2 free dims); l'=6,7 columns are garbage and
# never read.  The widest tap (c=2,d=2) then reads 2*S columns past the
# supergroup's 64*S image -- XPAD junk columns at the end of x cover it.
XPAD = 2 * SUPER


def _super_sizes():
    sizes = []
    left = PAIRS
    while left > 0:
        n = min(SUPER, left)
        sizes.append(n)
        left -= n
    return sizes


def build_w_stack(kern: np.ndarray) -> np.ndarray:
    """9 block-diagonal K[cd]*W stacks + 1 plain-W stack + K scales,
    concatenated as one [128, 9*72 + 72 + 9] bf16 weights image."""
    kern = np.asarray(kern, np.float32)
    W = np.zeros((64, 36), np.float32)
    for ip in range(6):
        for jp in range(6):
            m = ip * 6 + jp
            for a in range(3):
                for e in range(3):
                    W[(ip + a) * 8 + (jp + e), m] = kern[a, e]
    wstack = np.zeros((128, 9 * 72 + 72), np.float32)
    for s, (c, d) in enumerate(SHIFTS):
        wcd = kern[c, d] * W
        wstack[0:64, s * 72 : s * 72 + 36] = wcd
        wstack[64:128, s * 72 + 36 : s * 72 + 72] = wcd
    wstack[0:64, 648:684] = W
    wstack[64:128, 684:720] = W
    # K[cd] scales replicated per partition, f32 (tensor_scalar requires it)
    kscal = np.broadcast_to(kern.reshape(1, 9), (128, 9)).copy()
    return wstack.astype(NP_BF16), kscal


_PROGRAM_CACHE = {}


def build_program() -> bass.Bass:
    if "nc" in _PROGRAM_CACHE:
        return _PROGRAM_CACHE["nc"]

    # Bacc (not raw Bass): its compile()/finalize() runs
    # move_matmul_waits_to_ldweights + generate_event_semaphores, which split
    # multi-wait instructions (TRN2 allows 1 sync wait per instruction).
    nc = bacc.Bacc()
    x = nc.dram_tensor("x", [128, PAIRS * 64 + XPAD], BF16, kind="ExternalInput")
    w = nc.dram_tensor("w", [128, 9 * 72 + 72], BF16, kind="ExternalInput")
    k = nc.dram_tensor("k", [128, 9], F32, kind="ExternalInput")
    o = nc.dram_tensor("o", [72, PAIRS * 36], BF16, kind="ExternalOutput")

    with TileContext(nc) as tc:
        with (
            tc.tile_pool(name="wp", bufs=1) as wp,
            tc.tile_pool(name="xp", bufs=3) as xp,
            tc.tile_pool(name="zp", bufs=2) as zp,
            tc.tile_pool(name="pp", bufs=6, space="PSUM") as pp,
            tc.tile_pool(name="op", bufs=3) as op,
        ):
            wt = wp.tile([128, 9 * 72 + 72], BF16)
            nc.sync.dma_start(out=wt[:, :], in_=w[:, :])
            ktile = wp.tile([128, 9], F32)
            nc.sync.dma_start(out=ktile[:, :], in_=k[:, :])
            wplain = wt[:, 648:720]
            kt = ktile

            gidx = 0
            pcur = 0  # pair cursor
            for sidx, spairs in enumerate(_super_sizes()):
                path = ASSIGN[sidx % len(ASSIGN)]
                xg = xp.tile([128, SUPER * 64 + XPAD], BF16, tag="xg")
                nc.sync.dma_start(
                    out=xg[:, : spairs * 64 + XPAD],
                    in_=x[:, pcur * 64 : (pcur + spairs) * 64 + XPAD],
                )
                ot = op.tile([72, SUPER * 36], BF16, tag="ot")
                # SBUF image within a supergroup: [p][k(8)][l(8)][n].
                xv = xg[:, : spairs * 64].rearrange(
                    "p (k l n) -> p k l n", k=8, l=8
                )

                if path == "D":
                    # z[p, k', l8, n] = sum_cd K[cd] * x[p, k'+c, l8+d, n],
                    # l8 padded to 8 (l8=6,7 garbage, never read).  Taps are
                    # rank-3: [p][k'(6)][flat (l,n) 8S], bf16.  Native DVE
                    # ops only (mul to temp + add) -- the fused custom-ucode
                    # op measured 3x slower than native.
                    z16 = zp.tile([128, SUPER * 48], BF16, tag="z16")
                    tt = zp.tile([128, SUPER * 48], BF16, tag="tt")
                    S = spairs
                    zt = z16[:, : 48 * S].rearrange("p (k f) -> p k f", k=6)
                    tv = tt[:, : 48 * S].rearrange("p (k f) -> p k f", k=6)
                    for s, (c, d) in enumerate(SHIFTS):
                        # rows k'+c of the 8x8 image, cols l8+d wrapped:
                        # flat offset (8c+d)*S, 6 rows of stride 8S, 8S run
                        xw = xg[
                            :, (8 * c + d) * S : (8 * c + d) * S + 48 * S
                        ].rearrange("p (k f) -> p k f", k=6)
                        if s == 0:
                            nc.vector.tensor_scalar_mul(zt, xw, kt[:, 0:1])
                        else:
                            nc.vector.tensor_scalar_mul(tv, xw, kt[:, s : s + 1])
                            nc.vector.tensor_add(zt, zt, tv)
                    zmm = z16[:, : 48 * S].rearrange(
                        "p (k l n) -> p k l n", k=6, l=8
                    )

                done = 0
                while done < spairs:
                    npair = min(PAIRS_PER_GROUP, spairs - done)
                    nfree = npair * 36

                    ps = pp.tile([72, PAIRS_PER_GROUP * 36], F32, tag="ps")
                    # Gate matmul: absorbs the psum-slot-release (and, for
                    # group 0, the weight-DMA) wait so each real matmul
                    # carries at most one sync wait.
                    nc.tensor.matmul(
                        ps[0:2, 0:2], wt[:, 0:2], wt[:, 0:2], start=True, stop=True
                    )
                    if path == "P":
                        for s, (c, d) in enumerate(SHIFTS):
                            nc.tensor.matmul(
                                ps[:, :nfree],
                                wt[:, s * 72 : (s + 1) * 72],
                                xv[:, c : c + 6, d : d + 6, done : done + npair],
                                start=(s == 0),
                                stop=(s == len(SHIFTS) - 1),
                            )
                    else:
                        nc.tensor.matmul(
                            ps[:, :nfree],
                            wplain,
                            zmm[:, :, 0:6, done : done + npair],
                            start=True,
                            stop=True,
                        )

                    dst = ot[:, done * 36 : done * 36 + nfree]
                    nc.scalar.copy(out=dst, in_=ps[:, :nfree])
                    done += npair
                    gidx += 1

                nc.sync.dma_start(
                    out=o[:, pcur * 36 : (pcur + spairs) * 36],
                    in_=ot[:, : spairs * 36],
                )
                pcur += spairs

    # Bacc.finalize runs compile() (register alloc, wait splitting via event
    # semaphores) then freezes; the PJRT exec path requires a finalized nc.
    nc.finalize()

    _PROGRAM_CACHE["nc"] = nc
    return nc


def shard_inputs(input_tensor: np.ndarray, kern: np.ndarray):
    """Host prep: shuffle each core's slice into the SBUF image and downcast.

    Per supergroup of S pairs starting at pair P: partition p = 64*h + ij
    holds x[2*(P+n)+h, ij, k*8+l] at free offset P*64 + (k*8+l)*S + n.
    """
    x = np.ascontiguousarray(np.asarray(input_tensor, np.float32))
    xs = x.reshape(N_CORES, PAIRS, 2, 64, 64)  # (core, pair, h, ij, kl)
    wstack, kscal = build_w_stack(kern)
    in_maps = []
    for c in range(N_CORES):
        blocks = []
        pcur = 0
        for spairs in _super_sizes():
            blk = xs[c][pcur : pcur + spairs]          # (S, 2, 64, 64)
            blk = blk.transpose(1, 2, 3, 0)            # (2, ij, kl, S)
            blocks.append(blk.reshape(128, spairs * 64))
            pcur += spairs
        blocks.append(np.zeros((128, XPAD), np.float32))
        xd = np.concatenate(blocks, axis=1).astype(NP_BF16)
        in_maps.append({"x": np.ascontiguousarray(xd), "w": wstack, "k": kscal})
    return in_maps


def unshard_output(results) -> np.ndarray:
    """o[36*h+ij', (P+done)*36 + kl'*npair + n] -> out[b, i',j',k',l']."""
    outs = []
    for r in results:
        od = np.asarray(r["o"]).astype(np.float32)  # (72, PAIRS*36)
        out = np.empty((B_C, 36, 36), np.float32)
        pcur = 0
        for spairs in _super_sizes():
            done = 0
            while done < spairs:
                npair = min(PAIRS_PER_GROUP, spairs - done)
                col0 = (pcur + done) * 36
                blk = od[:, col0 : col0 + npair * 36]
                # rows (2, 36) x cols (36, npair) -> (npair, 2, ij', kl')
                blk = blk.reshape(2, 36, 36, npair).transpose(3, 0, 1, 2)
                b0 = (pcur + done) * 2
                out[b0 : b0 + 2 * npair] = blk.reshape(2 * npair, 36, 36)
                done += npair
            pcur += spairs
        outs.append(out.reshape(B_C, 6, 6, 6, 6))
    return np.concatenate(outs, axis=0)


def run(input_tensor: np.ndarray, kern: np.ndarray, **spmd_kwargs):
    """Shard, run on 8 cores, gather.  Returns (output, BassKernelResults)."""
    in_maps = shard_inputs(input_tensor, kern)
    nc = build_program()
    res = run_bass_kernel_spmd(nc, in_maps, core_ids=list(range(N_CORES)), **spmd_kwargs)
    return unshard_output(res.results), res


def kernel(input_tensor: np.ndarray, kernel: np.ndarray) -> np.ndarray:
    out, _ = run(input_tensor, kernel)
    return out
